# revision 11
# baseline (speedup 1.0000x reference)
"""TRN2 Bass kernel for nn_CompiledBlock_45148696216108 (moe_routing).

Reference computation:
    xp  = x[permute]
    xn  = LayerNorm(xp; gamma, beta, eps=1e-5)
    yp  = xn @ W.T + b
    out = (xp + yp)[argsort(permute)]

The block (LayerNorm + Linear + residual) is purely row-wise, so the
gather by `permute` and the scatter by its inverse cancel exactly:
    out = x + LN(x) @ W.T + b
No token movement (and no cross-core all-to-all) is needed. Tokens are
sharded contiguously across the 8 cores; the tiny weights are folded on
the host and replicated:
    A[h, o] = gamma[h] * W[o, h]          (pre-transposed, gamma folded)
    s[o]    = sum_h A[h, o]
    c[o]    = sum_h beta[h] * W[o, h] + b[o]

LayerNorm is affine per token, so it commutes with the matmul:
    out = x + q * (x @ A) + r * s + c,   q = rsqrt(var + eps), r = -mu * q
This lets the PE consume x directly - no on-device transpose of the
normalized activations is needed, because the host stages x a second
time in a transposed, PE-native tiling (pure layout/dtype prep):
    xtc[tile, hp, kh, tt] = x[tile*128 + tt, kh*128 + hp]  (bf16)
Each [128, 128] chunk of that layout IS the matmul stationary operand.

Per-core device pipeline (tokens_per_core = 8192, HIDDEN = 2048):
  - DMA x tile (128 tokens, 2048, fp32; residual + stats) and the
    matching xtc tile (bf16, transposed tiling)
  - DVE bn_stats/bn_aggr -> mean/var; ACT sqrt; DVE reciprocal -> q;
    DVE r = -mu*q; ACT rs = r*s; GPSIMD x += c; DVE x += rs
  - PE: 64 bf16 matmuls, k-outer so each stationary chunk is reused for
    4 consecutive instructions (keeps LDWEIGHTS off the critical path):
    psum_j[tt, o512] += xtc[kh].T @ A[kh][:, j*512:(j+1)*512]
  - DVE per bank: out_j = psum_j * q + (x + c + r*s)_j  (one
    scalar_tensor_tensor with per-partition scalar q); DMA out
8 PSUM banks rotate across tiles (4 banks x 2 buffers) so the next
tile's accumulation never waits on this tile's drain.
"""

import numpy as np
from contextlib import ExitStack

import ml_dtypes

from concourse import bacc, tile, mybir
from concourse.bass_utils import run_bass_kernel_spmd

N_TOK = 65536
HIDDEN = 2048
N_CORES = 8
P = 128
EPS = 1e-5
F32 = mybir.dt.float32
BF16 = mybir.dt.bfloat16
AF = mybir.ActivationFunctionType
ALU = mybir.AluOpType

NB = 512               # matmul moving free dim (one PSUM bank of fp32)
KC = HIDDEN // P       # 16 contraction chunks
OC = HIDDEN // NB      # 4 output column chunks


def build(tokens_per_core: int = N_TOK // N_CORES, num_devices: int = N_CORES):
    T = tokens_per_core
    NT = T // P            # token tiles

    nc = bacc.Bacc(
        "TRN2", target_bir_lowering=False, debug=False, num_devices=num_devices
    )
    x_d = nc.dram_tensor("x", [T, HIDDEN], F32, kind="ExternalInput").ap()
    xtc_d = nc.dram_tensor("xtc", [NT, P, HIDDEN], BF16, kind="ExternalInput").ap()
    a_d = nc.dram_tensor("A", [KC, OC, P, NB], BF16, kind="ExternalInput").ap()
    c_d = nc.dram_tensor("c", [P, HIDDEN], F32, kind="ExternalInput").ap()
    s_d = nc.dram_tensor("s", [P, HIDDEN], F32, kind="ExternalInput").ap()
    out_d = nc.dram_tensor("out", [T, HIDDEN], F32, kind="ExternalOutput").ap()

    with tile.TileContext(nc) as tc, ExitStack() as ctx:
        const = ctx.enter_context(tc.tile_pool(name="const", bufs=1))
        apool = ctx.enter_context(tc.tile_pool(name="apool", bufs=1))
        xpool = ctx.enter_context(tc.tile_pool(name="xpool", bufs=3))
        xtcpool = ctx.enter_context(tc.tile_pool(name="xtcpool", bufs=3))
        outpool = ctx.enter_context(tc.tile_pool(name="outpool", bufs=3))
        stpool = ctx.enter_context(tc.tile_pool(name="stats", bufs=3))
        rspool = ctx.enter_context(tc.tile_pool(name="rspool", bufs=2))
        psy_pool = ctx.enter_context(tc.tile_pool(name="psy", bufs=2, space="PSUM"))

        # Resident weights: 16 separately-tracked chunks of (128 h, 2048 o).
        # Each chunk is DMA'd as 4 column stripes issued k-major so chunks
        # COMPLETE in k order (16 whole-chunk DMAs on 16 queues would share
        # HBM bandwidth and all land together at ~20us); tile 0's k-outer
        # matmuls then stream right behind the weight load. The x/xtc/c/s
        # prefetches are interleaved sparsely so they don't starve chunk 0.
        xts, xtts = {}, {}
        a_sb = [
            apool.tile([P, HIDDEN], BF16, tag=f"a{k}", name=f"a_sb{k}")
            for k in range(KC)
        ]

        def dma_a(k):
            for j in range(OC):
                sl = slice(j * NB, (j + 1) * NB)
                nc.sync.dma_start(a_sb[k][:, sl], a_d[k][j])

        dma_a(0)
        xtts[0] = xtcpool.tile([P, HIDDEN], BF16, tag="xtt", name="xtt_pre0")
        nc.sync.dma_start(xtts[0][:], xtc_d[0])
        dma_a(1)
        xts[0] = xpool.tile([P, HIDDEN], F32, tag="xt", name="xt_pre0")
        nc.sync.dma_start(xts[0][:], x_d[0:P, :])
        dma_a(2)
        dma_a(3)
        c_sb = const.tile([P, HIDDEN], F32)
        nc.sync.dma_start(c_sb[:], c_d[:])
        dma_a(4)
        dma_a(5)
        s_sb = const.tile([P, HIDDEN], F32)
        nc.sync.dma_start(s_sb[:], s_d[:])
        for k in range(6, KC):
            dma_a(k)
        xtts[1] = xtcpool.tile([P, HIDDEN], BF16, tag="xtt", name="xtt_pre1")
        nc.sync.dma_start(xtts[1][:], xtc_d[1])
        xts[1] = xpool.tile([P, HIDDEN], F32, tag="xt", name="xt_pre1")
        nc.sync.dma_start(xts[1][:], x_d[P : 2 * P, :])
        eps_sb = const.tile([P, 1], F32)
        nc.gpsimd.memset(eps_sb[:], EPS)

        def prep(i):
            """Stats + residual pre-bias for tile i -> per-token scale q.

            Leaves xts[i] holding x + c + r*s (the stt addend) and returns
            the q tile.
            """
            xt = xts[i]
            stats = stpool.tile([P, 4, 6], F32, tag="stats")
            xr = xt[:].rearrange("p (a b) -> p a b", b=512)
            for a in range(4):
                nc.vector.bn_stats(stats[:, a, :], xr[:, a, :])
            mv = stpool.tile([P, 2], F32, tag="mv")
            nc.vector.bn_aggr(mv[:], stats[:])
            sig = stpool.tile([P, 1], F32, tag="sig")
            nc.scalar.activation(sig[:], mv[:, 1:2], AF.Sqrt, bias=eps_sb[:])
            q = stpool.tile([P, 1], F32, tag="q")
            nc.vector.reciprocal(q[:], sig[:])
            # p = mu*q on the scalar engine; s_sb holds -s so rs = -mu*q*s.
            p = stpool.tile([P, 1], F32, tag="p")
            nc.scalar.activation(p[:], mv[:, 0:1], AF.Identity, scale=q[:])
            rs = rspool.tile([P, HIDDEN], F32, tag="rs")
            nc.scalar.activation(rs[:], s_sb[:], AF.Identity, scale=p[:])
            # WAR on the bn_stats reads above: fold the constant bias and the
            # per-token -mu*q*s term into the residual in place.
            nc.gpsimd.tensor_add(xt[:], xt[:], c_sb[:])
            nc.gpsimd.tensor_add(xt[:], xt[:], rs[:])
            return q

        qs = {}
        qs[0] = prep(0)

        for t in range(NT):
            xt = xts.pop(t)
            xtt = xtts.pop(t)
            q = qs.pop(t)

            # Prefetch + prep next tile while this tile's matmuls run
            if t + 2 < NT:
                xts[t + 2] = xpool.tile([P, HIDDEN], F32, tag="xt", name=f"xt_{t + 2}")
                nc.sync.dma_start(
                    xts[t + 2][:], x_d[(t + 2) * P : (t + 3) * P, :]
                )
                xtts[t + 2] = xtcpool.tile(
                    [P, HIDDEN], BF16, tag="xtt", name=f"xtt_{t + 2}"
                )
                nc.sync.dma_start(xtts[t + 2][:], xtc_d[t + 2])
            if t + 1 < NT:
                qs[t + 1] = prep(t + 1)

            # Matmuls: k-outer so the stationary operand (a 128x128 chunk of
            # the transposed x tile) is reused by 4 consecutive matmuls, one
            # per PSUM bank; all 4 banks accumulate across the kh sweep.
            # The LAST tile runs j-outer with per-bank combine + per-stripe
            # store instead, so the tail drain overlaps its final matmuls.
            last = t == NT - 1
            psys = [
                psy_pool.tile([P, NB], F32, tag=f"psy{j}", name=f"psy_{t}_{j}")
                for j in range(OC)
            ]
            ot = outpool.tile([P, HIDDEN], F32, tag="ot")
            # Tile 0 consumes chunks in rotated order starting at G: the PE
            # then begins only once chunk G's DMA lands (~60% into the
            # weight stream) and streams gap-free behind the remaining
            # chunks -- starting earlier would trickle-stall, and every
            # stall resets the PE p-state ramp (post-stall matmuls run at
            # the mid p-state until ~3us of continuous execution).
            G = 8
            kseq = (
                list(range(G, KC)) + list(range(G)) if t == 0 else list(range(KC))
            )
            order = (
                [(j, k) for j in range(OC) for k in kseq]
                if last
                else [(j, k) for k in kseq for j in range(OC)]
            )
            for j, k in order:
                nc.tensor.matmul(
                    psys[j][:],
                    xtt[:, k * P : (k + 1) * P],
                    a_sb[k][:, j * NB : (j + 1) * NB],
                    start=(k == kseq[0]),
                    stop=(k == kseq[-1]),
                )
                if last and k == KC - 1:
                    sl = slice(j * NB, (j + 1) * NB)
                    nc.vector.scalar_tensor_tensor(
                        ot[:, sl], psys[j][:], q[:], xt[:, sl], ALU.mult, ALU.add
                    )
                    nc.sync.dma_start(
                        out_d[t * P : (t + 1) * P, sl], ot[:, sl]
                    )
            if not last:
                for j in range(OC):
                    sl = slice(j * NB, (j + 1) * NB)
                    nc.vector.scalar_tensor_tensor(
                        ot[:, sl], psys[j][:], q[:], xt[:, sl], ALU.mult, ALU.add
                    )
                nc.sync.dma_start(out_d[t * P : (t + 1) * P, :], ot[:])

    nc.compile()
    return nc


_built = None


def _get_built():
    global _built
    if _built is None:
        _built = build()
    return _built


def _prep_inputs(x, permute, gamma, beta, W, b):
    x = np.asarray(x, dtype=np.float32)
    gamma = np.asarray(gamma, dtype=np.float64)
    beta = np.asarray(beta, dtype=np.float64)
    W = np.asarray(W, dtype=np.float64)
    b = np.asarray(b, dtype=np.float64)
    A = W.T * gamma[:, None]                      # (H, O), gamma folded
    A_bf = A.astype(np.float32).astype(ml_dtypes.bfloat16)
    s = -A_bf.astype(np.float64).sum(axis=0)      # negated; matches bf16 matmul
    # [KC, OC, P, NB]: each (chunk, output-stripe) block is contiguous so
    # the per-stripe weight DMAs are dense.
    A_bf = np.ascontiguousarray(
        A_bf.reshape(KC, P, OC, NB).transpose(0, 2, 1, 3)
    )
    c = np.ascontiguousarray(
        np.broadcast_to((W @ beta + b).reshape(1, HIDDEN), (P, HIDDEN))
    ).astype(np.float32)
    s = np.ascontiguousarray(
        np.broadcast_to(s.reshape(1, HIDDEN), (P, HIDDEN))
    ).astype(np.float32)
    T = N_TOK // N_CORES
    NT = T // P
    in_maps = []
    for i in range(N_CORES):
        xs = x[i * T : (i + 1) * T]
        # Transposed, PE-native tiling: xtc[tile, hp, kh, tt] =
        # xs[tile*128 + tt, kh*128 + hp]; chunk kh of a tile is the matmul
        # stationary operand [K=hp, M=tt].
        xtc = np.ascontiguousarray(
            xs.reshape(NT, P, KC, P).transpose(0, 3, 2, 1)
        ).astype(ml_dtypes.bfloat16).reshape(NT, P, HIDDEN)
        in_maps.append({"x": xs, "xtc": xtc, "A": A_bf, "c": c, "s": s})
    return in_maps


def kernel(x, permute, gamma, beta, W, b):
    nc = _get_built()
    in_maps = _prep_inputs(x, permute, gamma, beta, W, b)
    res = run_bass_kernel_spmd(nc, in_maps, list(range(N_CORES))).results
    return np.concatenate([r["out"] for r in res], axis=0)


if __name__ == "__main__":
    rng = np.random.default_rng(0)
    x = rng.standard_normal((N_TOK, HIDDEN), dtype=np.float32)
    permute = rng.permutation(N_TOK).astype(np.int64)
    gamma = np.ones(HIDDEN, np.float32)
    beta = np.zeros(HIDDEN, np.float32)
    W = (rng.standard_normal((HIDDEN, HIDDEN), dtype=np.float32) / np.sqrt(HIDDEN))
    b = rng.standard_normal(HIDDEN, dtype=np.float32) * 0.01
    out = kernel(x=x, permute=permute, gamma=gamma, beta=beta, W=W, b=b)
    print(out.shape, out.dtype)


# revision 12
# speedup vs baseline: 1.0329x; 1.0329x over previous
"""TRN2 Bass kernel for nn_CompiledBlock_45148696216108 (moe_routing).

Reference computation:
    xp  = x[permute]
    xn  = LayerNorm(xp; gamma, beta, eps=1e-5)
    yp  = xn @ W.T + b
    out = (xp + yp)[argsort(permute)]

The block (LayerNorm + Linear + residual) is purely row-wise, so the
gather by `permute` and the scatter by its inverse cancel exactly:
    out = x + LN(x) @ W.T + b
No token movement (and no cross-core all-to-all) is needed. Tokens are
sharded contiguously across the 8 cores; the tiny weights are folded on
the host and replicated:
    A[h, o] = gamma[h] * W[o, h]          (pre-transposed, gamma folded)
    s[o]    = sum_h A[h, o]
    c[o]    = sum_h beta[h] * W[o, h] + b[o]

LayerNorm is affine per token, so it commutes with the matmul:
    out = x + q * (x @ A) + r * s + c,   q = rsqrt(var + eps), r = -mu * q
This lets the PE consume x directly - no on-device transpose of the
normalized activations is needed, because the host stages x a second
time in a transposed, PE-native tiling (pure layout/dtype prep):
    xtc[tile, hp, kh, tt] = x[tile*128 + tt, kh*128 + hp]  (bf16)
Each [128, 128] chunk of that layout IS the matmul stationary operand.

Per-core device pipeline (tokens_per_core = 8192, HIDDEN = 2048):
  - DMA x tile (128 tokens, 2048, fp32; residual + stats) and the
    matching xtc tile (bf16, transposed tiling)
  - DVE bn_stats/bn_aggr -> mean/var; ACT sqrt; DVE reciprocal -> q;
    DVE r = -mu*q; ACT rs = r*s; GPSIMD x += c; DVE x += rs
  - PE: 64 bf16 matmuls, k-outer so each stationary chunk is reused for
    4 consecutive instructions (keeps LDWEIGHTS off the critical path):
    psum_j[tt, o512] += xtc[kh].T @ A[kh][:, j*512:(j+1)*512]
  - DVE per bank: out_j = psum_j * q + (x + c + r*s)_j  (one
    scalar_tensor_tensor with per-partition scalar q); DMA out
8 PSUM banks rotate across tiles (4 banks x 2 buffers) so the next
tile's accumulation never waits on this tile's drain.
"""

import numpy as np
from contextlib import ExitStack

import ml_dtypes

from concourse import bacc, tile, mybir
from concourse.bass_utils import run_bass_kernel_spmd

N_TOK = 65536
HIDDEN = 2048
N_CORES = 8
P = 128
EPS = 1e-5
F32 = mybir.dt.float32
BF16 = mybir.dt.bfloat16
F8E3 = mybir.dt.float8e3
AF = mybir.ActivationFunctionType
ALU = mybir.AluOpType

NB = 512               # matmul moving free dim (one PSUM bank of fp32)
KC = HIDDEN // P       # 16 contraction chunks
OC = HIDDEN // NB      # 4 output column chunks


def build(tokens_per_core: int = N_TOK // N_CORES, num_devices: int = N_CORES):
    T = tokens_per_core
    NT = T // P            # token tiles

    nc = bacc.Bacc(
        "TRN2", target_bir_lowering=False, debug=False, num_devices=num_devices
    )
    x_d = nc.dram_tensor("x", [T, HIDDEN], F32, kind="ExternalInput").ap()
    xtc_d = nc.dram_tensor("xtc", [NT, P, HIDDEN], BF16, kind="ExternalInput").ap()
    a_d = nc.dram_tensor("A", [KC, P, HIDDEN], F8E3, kind="ExternalInput").ap()
    c_d = nc.dram_tensor("c", [P, HIDDEN], F32, kind="ExternalInput").ap()
    s_d = nc.dram_tensor("s", [P, HIDDEN], F32, kind="ExternalInput").ap()
    out_d = nc.dram_tensor("out", [T, HIDDEN], F32, kind="ExternalOutput").ap()

    with tile.TileContext(nc) as tc, ExitStack() as ctx:
        const = ctx.enter_context(tc.tile_pool(name="const", bufs=1))
        apool = ctx.enter_context(tc.tile_pool(name="apool", bufs=1))
        xpool = ctx.enter_context(tc.tile_pool(name="xpool", bufs=3))
        xtcpool = ctx.enter_context(tc.tile_pool(name="xtcpool", bufs=3))
        outpool = ctx.enter_context(tc.tile_pool(name="outpool", bufs=3))
        stpool = ctx.enter_context(tc.tile_pool(name="stats", bufs=3))
        rspool = ctx.enter_context(tc.tile_pool(name="rspool", bufs=2))
        psy_pool = ctx.enter_context(tc.tile_pool(name="psy", bufs=2, space="PSUM"))

        # Resident weights: 16 separately-tracked chunks of (128 h, 2048 o)
        # in fp8e3 (e3m4, x64 host-scaled), halving the startup weight DMA
        # vs bf16. One whole-chunk DMA each (multiple writers per tile add
        # per-matmul sync overhead). The first matmul needs only xtt[0] +
        # chunk 0, so those are issued first.
        xts, xtts = {}, {}
        a_sb = [
            apool.tile([P, HIDDEN], F8E3, tag=f"a{k}", name=f"a_sb{k}")
            for k in range(KC)
        ]
        xtts[0] = xtcpool.tile([P, HIDDEN], BF16, tag="xtt", name="xtt_pre0")
        nc.sync.dma_start(xtts[0][:], xtc_d[0])
        for k in range(KC):
            nc.sync.dma_start(a_sb[k][:], a_d[k])
        xts[0] = xpool.tile([P, HIDDEN], F32, tag="xt", name="xt_pre0")
        nc.sync.dma_start(xts[0][:], x_d[0:P, :])
        c_sb = const.tile([P, HIDDEN], F32)
        nc.sync.dma_start(c_sb[:], c_d[:])
        s_sb = const.tile([P, HIDDEN], F32)
        nc.sync.dma_start(s_sb[:], s_d[:])
        xtts[1] = xtcpool.tile([P, HIDDEN], BF16, tag="xtt", name="xtt_pre1")
        nc.sync.dma_start(xtts[1][:], xtc_d[1])
        xts[1] = xpool.tile([P, HIDDEN], F32, tag="xt", name="xt_pre1")
        nc.sync.dma_start(xts[1][:], x_d[P : 2 * P, :])
        eps_sb = const.tile([P, 1], F32)
        nc.gpsimd.memset(eps_sb[:], EPS * 4096.0)

        def prep(i):
            """Stats + residual pre-bias for tile i -> per-token scale q.

            Leaves xts[i] holding x + c + r*s (the stt addend) and returns
            the q tile.
            """
            xt = xts[i]
            stats = stpool.tile([P, 4, 6], F32, tag="stats")
            xr = xt[:].rearrange("p (a b) -> p a b", b=512)
            for a in range(4):
                nc.vector.bn_stats(stats[:, a, :], xr[:, a, :])
            mv = stpool.tile([P, 2], F32, tag="mv")
            nc.vector.bn_aggr(mv[:], stats[:])
            sig = stpool.tile([P, 1], F32, tag="sig")
            # sig = 64*sqrt(var+eps): the 1/64 compensating the x64
            # host-scaled fp8 weights rides along in q = 1/sig; the
            # rs path uses s_sb = -64*s so it cancels there too.
            nc.scalar.activation(
                sig[:], mv[:, 1:2], AF.Sqrt, bias=eps_sb[:], scale=4096.0
            )
            q = stpool.tile([P, 1], F32, tag="q")
            nc.vector.reciprocal(q[:], sig[:])
            # p = mu*q on the scalar engine; s_sb holds -s so rs = -mu*q*s.
            p = stpool.tile([P, 1], F32, tag="p")
            nc.scalar.activation(p[:], mv[:, 0:1], AF.Identity, scale=q[:])
            rs = rspool.tile([P, HIDDEN], F32, tag="rs")
            nc.scalar.activation(rs[:], s_sb[:], AF.Identity, scale=p[:])
            # WAR on the bn_stats reads above: fold the constant bias and the
            # per-token -mu*q*s term into the residual in place.
            nc.gpsimd.tensor_add(xt[:], xt[:], c_sb[:])
            nc.gpsimd.tensor_add(xt[:], xt[:], rs[:])
            return q

        qs = {}
        qs[0] = prep(0)

        for t in range(NT):
            xt = xts.pop(t)
            xtt = xtts.pop(t)
            q = qs.pop(t)

            # Prefetch + prep next tile while this tile's matmuls run
            if t + 2 < NT:
                xts[t + 2] = xpool.tile([P, HIDDEN], F32, tag="xt", name=f"xt_{t + 2}")
                nc.sync.dma_start(
                    xts[t + 2][:], x_d[(t + 2) * P : (t + 3) * P, :]
                )
                xtts[t + 2] = xtcpool.tile(
                    [P, HIDDEN], BF16, tag="xtt", name=f"xtt_{t + 2}"
                )
                nc.sync.dma_start(xtts[t + 2][:], xtc_d[t + 2])
            if t + 1 < NT:
                qs[t + 1] = prep(t + 1)

            # Matmuls: k-outer so the stationary operand (a 128x128 chunk of
            # the transposed x tile) is reused by 4 consecutive matmuls, one
            # per PSUM bank; all 4 banks accumulate across the kh sweep.
            # The LAST tile runs j-outer with per-bank combine + per-stripe
            # store instead, so the tail drain overlaps its final matmuls.
            last = t == NT - 1
            psys = [
                psy_pool.tile([P, NB], F32, tag=f"psy{j}", name=f"psy_{t}_{j}")
                for j in range(OC)
            ]
            ot = outpool.tile([P, HIDDEN], F32, tag="ot")
            order = (
                [(j, k) for j in range(OC) for k in range(KC)]
                if last
                else [(j, k) for k in range(KC) for j in range(OC)]
            )
            for j, k in order:
                nc.tensor.matmul(
                    psys[j][:],
                    xtt[:, k * P : (k + 1) * P],
                    a_sb[k][:, j * NB : (j + 1) * NB],
                    start=(k == 0),
                    stop=(k == KC - 1),
                )
                if last and k == KC - 1:
                    sl = slice(j * NB, (j + 1) * NB)
                    nc.vector.scalar_tensor_tensor(
                        ot[:, sl], psys[j][:], q[:], xt[:, sl], ALU.mult, ALU.add
                    )
                    nc.sync.dma_start(
                        out_d[t * P : (t + 1) * P, sl], ot[:, sl]
                    )
            if not last:
                for j in range(OC):
                    sl = slice(j * NB, (j + 1) * NB)
                    nc.vector.scalar_tensor_tensor(
                        ot[:, sl], psys[j][:], q[:], xt[:, sl], ALU.mult, ALU.add
                    )
                nc.sync.dma_start(out_d[t * P : (t + 1) * P, :], ot[:])

    nc.compile()
    return nc


_built = None


def _get_built():
    global _built
    if _built is None:
        _built = build()
    return _built


def _prep_inputs(x, permute, gamma, beta, W, b):
    x = np.asarray(x, dtype=np.float32)
    gamma = np.asarray(gamma, dtype=np.float64)
    beta = np.asarray(beta, dtype=np.float64)
    W = np.asarray(W, dtype=np.float64)
    b = np.asarray(b, dtype=np.float64)
    A = W.T * gamma[:, None]                      # (H, O), gamma folded
    # Weights in fp8e3 (e3m4), host-scaled by 64 into the format's normal
    # range; the kernel's q = 1/(64*sqrt(var+eps)) compensates. s carries
    # the x64 (and the minus) so the rs path needs no extra device ops.
    A_q = (A * 64.0).astype(np.float32).astype(ml_dtypes.float8_e3m4)
    s = -A_q.astype(np.float64).sum(axis=0)       # = -64*sum_h A[h,o]
    A_bf = np.ascontiguousarray(A_q.reshape(KC, P, HIDDEN))
    c = np.ascontiguousarray(
        np.broadcast_to((W @ beta + b).reshape(1, HIDDEN), (P, HIDDEN))
    ).astype(np.float32)
    s = np.ascontiguousarray(
        np.broadcast_to(s.reshape(1, HIDDEN), (P, HIDDEN))
    ).astype(np.float32)
    T = N_TOK // N_CORES
    NT = T // P
    in_maps = []
    for i in range(N_CORES):
        xs = x[i * T : (i + 1) * T]
        # Transposed, PE-native tiling: xtc[tile, hp, kh, tt] =
        # xs[tile*128 + tt, kh*128 + hp]; chunk kh of a tile is the matmul
        # stationary operand [K=hp, M=tt].
        xtc = np.ascontiguousarray(
            xs.reshape(NT, P, KC, P).transpose(0, 3, 2, 1)
        ).astype(ml_dtypes.bfloat16).reshape(NT, P, HIDDEN)
        in_maps.append({"x": xs, "xtc": xtc, "A": A_bf, "c": c, "s": s})
    return in_maps


def kernel(x, permute, gamma, beta, W, b):
    nc = _get_built()
    in_maps = _prep_inputs(x, permute, gamma, beta, W, b)
    res = run_bass_kernel_spmd(nc, in_maps, list(range(N_CORES))).results
    return np.concatenate([r["out"] for r in res], axis=0)


if __name__ == "__main__":
    rng = np.random.default_rng(0)
    x = rng.standard_normal((N_TOK, HIDDEN), dtype=np.float32)
    permute = rng.permutation(N_TOK).astype(np.int64)
    gamma = np.ones(HIDDEN, np.float32)
    beta = np.zeros(HIDDEN, np.float32)
    W = (rng.standard_normal((HIDDEN, HIDDEN), dtype=np.float32) / np.sqrt(HIDDEN))
    b = rng.standard_normal(HIDDEN, dtype=np.float32) * 0.01
    out = kernel(x=x, permute=permute, gamma=gamma, beta=beta, W=W, b=b)
    print(out.shape, out.dtype)


# revision 14
# speedup vs baseline: 1.1245x; 1.0887x over previous
"""TRN2 Bass kernel for nn_CompiledBlock_45148696216108 (moe_routing).

Reference computation:
    xp  = x[permute]
    xn  = LayerNorm(xp; gamma, beta, eps=1e-5)
    yp  = xn @ W.T + b
    out = (xp + yp)[argsort(permute)]

The block (LayerNorm + Linear + residual) is purely row-wise, so the
gather by `permute` and the scatter by its inverse cancel exactly:
    out = x + LN(x) @ W.T + b
No token movement (and no cross-core all-to-all) is needed. Tokens are
sharded contiguously across the 8 cores; the tiny weights are folded on
the host and replicated:
    A[h, o] = gamma[h] * W[o, h]          (pre-transposed, gamma folded)
    s[o]    = sum_h A[h, o]
    c[o]    = sum_h beta[h] * W[o, h] + b[o]

LayerNorm is affine per token, so it commutes with the matmul:
    out = x + q * (x @ A) + r * s + c,   q = rsqrt(var + eps), r = -mu * q
This lets the PE consume x directly - no on-device transpose of the
normalized activations is needed, because the host stages x a second
time in a transposed, PE-native tiling (pure layout/dtype prep):
    xtc[tile, hp, kh, tt] = x[tile*128 + tt, kh*128 + hp]
Each [128, 128] chunk of that layout IS the matmul stationary operand.

Contraction runs in mixed precision: 14 of the 16 k-chunks in bf16
(1 cycle/row) and the last 2 chunks as a single fp8-e4m3 DoubleRow
matmul (0.5 cycles/row, 2 k-chunks per instruction), cutting PE work
~9% for ~1.3e-2 relative error (gate is 2e-2). All weights are host-
scaled by 64 (exact in bf16, centers fp8 in its normal range); the
on-device q absorbs the 1/64 via sqrt(4096*(var+eps)).

Per-core device pipeline (tokens_per_core = 8192, HIDDEN = 2048):
  - DMA x tile (128 tokens, fp32; residual + stats), xtc tile (bf16,
    transposed tiling, 14 chunks) and xtc8 tile (e4m3, 2 chunks)
  - DVE bn_stats/bn_aggr -> mean/var; ACT sig=64*sqrt(var+eps); DVE
    q=1/sig; ACT p=mu*q, rs=p*(-64s); GPSIMD x+=c, x+=rs
  - PE per bank j: 14 bf16 matmuls + 1 DoubleRow fp8 matmul, k-outer so
    each stationary chunk is reused by 4 consecutive instructions
  - DVE per bank: out_j = psum_j * q + (x + c + r*s)_j  (one
    scalar_tensor_tensor with per-partition scalar q); DMA out
8 PSUM banks rotate across tiles (4 banks x 2 buffers) so the next
tile's accumulation never waits on this tile's drain. The last tile
runs j-outer with per-bank combine + store so the tail overlaps.
"""

import numpy as np
from contextlib import ExitStack

import ml_dtypes

from concourse import bacc, tile, mybir
from concourse.bass_utils import run_bass_kernel_spmd

N_TOK = 65536
HIDDEN = 2048
N_CORES = 8
P = 128
EPS = 1e-5
F32 = mybir.dt.float32
BF16 = mybir.dt.bfloat16
F8E4 = mybir.dt.float8e4
AF = mybir.ActivationFunctionType
ALU = mybir.AluOpType
DR = mybir.MatmulPerfMode.DoubleRow

NB = 512               # matmul moving free dim (one PSUM bank of fp32)
KC = HIDDEN // P       # 16 contraction chunks
KM = 14                # chunks done in bf16; the last KC-KM go fp8 DoubleRow
OC = HIDDEN // NB      # 4 output column chunks
WSCALE = 64.0          # host weight scale; q absorbs the inverse


def build(tokens_per_core: int = N_TOK // N_CORES, num_devices: int = N_CORES):
    T = tokens_per_core
    NT = T // P            # token tiles

    nc = bacc.Bacc(
        "TRN2", target_bir_lowering=False, debug=False, num_devices=num_devices
    )
    x_d = nc.dram_tensor("x", [T, HIDDEN], F32, kind="ExternalInput").ap()
    xtc_d = nc.dram_tensor("xtc", [NT, P, KM * P], BF16, kind="ExternalInput").ap()
    xtc8_d = nc.dram_tensor(
        "xtc8", [NT, P, (KC - KM) * P], F8E4, kind="ExternalInput"
    ).ap()
    a_d = nc.dram_tensor("A", [KM, P, HIDDEN], BF16, kind="ExternalInput").ap()
    a8_d = nc.dram_tensor("A8", [P, KC - KM, HIDDEN], F8E4, kind="ExternalInput").ap()
    c_d = nc.dram_tensor("c", [P, HIDDEN], F32, kind="ExternalInput").ap()
    s_d = nc.dram_tensor("s", [P, HIDDEN], F32, kind="ExternalInput").ap()
    out_d = nc.dram_tensor("out", [T, HIDDEN], F32, kind="ExternalOutput").ap()

    with tile.TileContext(nc) as tc, ExitStack() as ctx:
        const = ctx.enter_context(tc.tile_pool(name="const", bufs=1))
        apool = ctx.enter_context(tc.tile_pool(name="apool", bufs=1))
        xpool = ctx.enter_context(tc.tile_pool(name="xpool", bufs=3))
        xtcpool = ctx.enter_context(tc.tile_pool(name="xtcpool", bufs=3))
        outpool = ctx.enter_context(tc.tile_pool(name="outpool", bufs=3))
        stpool = ctx.enter_context(tc.tile_pool(name="stats", bufs=3))
        rspool = ctx.enter_context(tc.tile_pool(name="rspool", bufs=2))
        x8pool = ctx.enter_context(tc.tile_pool(name="x8pool", bufs=3))
        a8pool = ctx.enter_context(tc.tile_pool(name="a8pool", bufs=1))
        psy_pool = ctx.enter_context(tc.tile_pool(name="psy", bufs=2, space="PSUM"))

        # Resident weights, one whole-chunk DMA each (separately tracked so
        # matmuls on chunk k only wait for chunk k's DMA; multiple writers
        # per tile would add per-matmul sync overhead). The first matmul
        # needs only xtt[0] + chunk 0, so those are issued first.
        xts, xtts, xtt8s = {}, {}, {}
        a_sb = [
            apool.tile([P, HIDDEN], BF16, tag=f"a{k}", name=f"a_sb{k}")
            for k in range(KM)
        ]
        xtts[0] = xtcpool.tile([P, KM * P], BF16, tag="xtt", name="xtt_pre0")
        nc.sync.dma_start(xtts[0][:], xtc_d[0])
        xtt8s[0] = x8pool.tile(
            [P, (KC - KM) * P], F8E4, tag="xtt8", name="xtt8_pre0"
        )
        nc.sync.dma_start(xtt8s[0][:], xtc8_d[0])
        for k in range(KM):
            nc.sync.dma_start(a_sb[k][:], a_d[k])
        a8_sb = a8pool.tile([P, KC - KM, HIDDEN], F8E4, tag="a8")
        nc.sync.dma_start(a8_sb[:], a8_d[:])
        xts[0] = xpool.tile([P, HIDDEN], F32, tag="xt", name="xt_pre0")
        nc.sync.dma_start(xts[0][:], x_d[0:P, :])
        c_sb = const.tile([P, HIDDEN], F32)
        nc.sync.dma_start(c_sb[:], c_d[:])
        s_sb = const.tile([P, HIDDEN], F32)
        nc.sync.dma_start(s_sb[:], s_d[:])
        xtts[1] = xtcpool.tile([P, KM * P], BF16, tag="xtt", name="xtt_pre1")
        nc.sync.dma_start(xtts[1][:], xtc_d[1])
        xtt8s[1] = x8pool.tile(
            [P, (KC - KM) * P], F8E4, tag="xtt8", name="xtt8_pre1"
        )
        nc.sync.dma_start(xtt8s[1][:], xtc8_d[1])
        xts[1] = xpool.tile([P, HIDDEN], F32, tag="xt", name="xt_pre1")
        nc.sync.dma_start(xts[1][:], x_d[P : 2 * P, :])
        eps_sb = const.tile([P, 1], F32)
        nc.gpsimd.memset(eps_sb[:], EPS * WSCALE * WSCALE)

        def prep(i):
            """Stats + residual pre-bias for tile i -> per-token scale q.

            Leaves xts[i] holding x + c + r*s (the stt addend) and returns
            the q tile (which carries the 1/WSCALE weight compensation).
            """
            xt = xts[i]
            stats = stpool.tile([P, 4, 6], F32, tag="stats")
            xr = xt[:].rearrange("p (a b) -> p a b", b=512)
            for a in range(4):
                nc.vector.bn_stats(stats[:, a, :], xr[:, a, :])
            mv = stpool.tile([P, 2], F32, tag="mv")
            nc.vector.bn_aggr(mv[:], stats[:])
            sig = stpool.tile([P, 1], F32, tag="sig")
            # sig = WSCALE*sqrt(var+eps): the 1/WSCALE compensating the
            # host-scaled weights rides along in q = 1/sig; the rs path
            # uses s_sb = -WSCALE*s so it cancels there too.
            nc.scalar.activation(
                sig[:], mv[:, 1:2], AF.Sqrt, bias=eps_sb[:], scale=WSCALE * WSCALE
            )
            q = stpool.tile([P, 1], F32, tag="q")
            nc.vector.reciprocal(q[:], sig[:])
            p = stpool.tile([P, 1], F32, tag="p")
            nc.scalar.activation(p[:], mv[:, 0:1], AF.Identity, scale=q[:])
            rs = rspool.tile([P, HIDDEN], F32, tag="rs")
            nc.scalar.activation(rs[:], s_sb[:], AF.Identity, scale=p[:])
            # WAR on the bn_stats reads above: fold the constant bias and the
            # per-token -mu*q*s term into the residual in place.
            nc.gpsimd.tensor_add(xt[:], xt[:], c_sb[:])
            nc.gpsimd.tensor_add(xt[:], xt[:], rs[:])
            return q

        qs = {}
        qs[0] = prep(0)

        for t in range(NT):
            xt = xts.pop(t)
            xtt = xtts.pop(t)
            xtt8 = xtt8s.pop(t)
            q = qs.pop(t)

            # Prefetch + prep next tile while this tile's matmuls run
            if t + 2 < NT:
                xts[t + 2] = xpool.tile([P, HIDDEN], F32, tag="xt", name=f"xt_{t + 2}")
                nc.sync.dma_start(
                    xts[t + 2][:], x_d[(t + 2) * P : (t + 3) * P, :]
                )
                xtts[t + 2] = xtcpool.tile(
                    [P, KM * P], BF16, tag="xtt", name=f"xtt_{t + 2}"
                )
                nc.sync.dma_start(xtts[t + 2][:], xtc_d[t + 2])
                xtt8s[t + 2] = x8pool.tile(
                    [P, (KC - KM) * P], F8E4, tag="xtt8", name=f"xtt8_{t + 2}"
                )
                nc.sync.dma_start(xtt8s[t + 2][:], xtc8_d[t + 2])
            if t + 1 < NT:
                qs[t + 1] = prep(t + 1)

            # Matmuls: k-outer so the stationary operand (a 128x128 chunk of
            # the transposed x tile) is reused by 4 consecutive matmuls, one
            # per PSUM bank. 14 bf16 chunks, then one fp8 DoubleRow matmul
            # per bank contracts the last 2 chunks at half cost. The LAST
            # tile runs j-outer with per-bank combine + per-stripe store
            # instead, so the tail drain overlaps its final matmuls.
            last = t == NT - 1
            psys = [
                psy_pool.tile([P, NB], F32, tag=f"psy{j}", name=f"psy_{t}_{j}")
                for j in range(OC)
            ]
            ot = outpool.tile([P, HIDDEN], F32, tag="ot")
            xtt8_ap = xtt8[:].rearrange("p (two f) -> p two f", two=KC - KM)

            def mm(j, k):
                if k < KM:
                    nc.tensor.matmul(
                        psys[j][:],
                        xtt[:, k * P : (k + 1) * P],
                        a_sb[k][:, j * NB : (j + 1) * NB],
                        start=(k == 0),
                        stop=False,
                    )
                else:
                    nc.tensor.matmul(
                        psys[j][:],
                        xtt8_ap,
                        a8_sb[:, :, j * NB : (j + 1) * NB],
                        start=False,
                        stop=True,
                        perf_mode=DR,
                    )

            order = (
                [(j, k) for j in range(OC) for k in range(KM + 1)]
                if last
                else [(j, k) for k in range(KM + 1) for j in range(OC)]
            )
            for j, k in order:
                mm(j, k)
                if last and k == KM:
                    sl = slice(j * NB, (j + 1) * NB)
                    nc.vector.scalar_tensor_tensor(
                        ot[:, sl], psys[j][:], q[:], xt[:, sl], ALU.mult, ALU.add
                    )
                    nc.sync.dma_start(out_d[t * P : (t + 1) * P, sl], ot[:, sl])
            if not last:
                for j in range(OC):
                    sl = slice(j * NB, (j + 1) * NB)
                    nc.vector.scalar_tensor_tensor(
                        ot[:, sl], psys[j][:], q[:], xt[:, sl], ALU.mult, ALU.add
                    )
                nc.sync.dma_start(out_d[t * P : (t + 1) * P, :], ot[:])

    nc.compile()
    return nc


_built = None


def _get_built():
    global _built
    if _built is None:
        _built = build()
    return _built


def _prep_inputs(x, permute, gamma, beta, W, b):
    x = np.asarray(x, dtype=np.float32)
    gamma = np.asarray(gamma, dtype=np.float64)
    beta = np.asarray(beta, dtype=np.float64)
    W = np.asarray(W, dtype=np.float64)
    b = np.asarray(b, dtype=np.float64)
    A = W.T * gamma[:, None]                      # (H, O), gamma folded
    # All weights host-scaled by WSCALE (exact in bf16, centers fp8e4m3 in
    # its normal range); the kernel's q = 1/(WSCALE*sqrt(var+eps))
    # compensates. Chunks 0..KM-1 in bf16, the rest in e4m3 (DoubleRow).
    A_sc = (A * WSCALE).astype(np.float32).reshape(KC, P, HIDDEN)
    A_bf = A_sc[:KM].astype(ml_dtypes.bfloat16)
    A_f8 = np.ascontiguousarray(
        A_sc[KM:].transpose(1, 0, 2)              # [P, KC-KM, HIDDEN]
    ).astype(ml_dtypes.float8_e4m3)
    # s from the values the device will actually multiply (and negated;
    # carries the WSCALE so the rs path needs no extra device ops).
    s = -(
        A_bf.astype(np.float64).sum(axis=(0, 1))
        + A_f8.astype(np.float64).sum(axis=(0, 1))
    )
    c = np.ascontiguousarray(
        np.broadcast_to((W @ beta + b).reshape(1, HIDDEN), (P, HIDDEN))
    ).astype(np.float32)
    s = np.ascontiguousarray(
        np.broadcast_to(s.reshape(1, HIDDEN), (P, HIDDEN))
    ).astype(np.float32)
    T = N_TOK // N_CORES
    NT = T // P
    in_maps = []
    for i in range(N_CORES):
        xs = x[i * T : (i + 1) * T]
        # Transposed, PE-native tiling: xtc[tile, hp, kh, tt] =
        # xs[tile*128 + tt, kh*128 + hp]; chunk kh of a tile is the matmul
        # stationary operand [K=hp, M=tt]. bf16 for the first KM chunks,
        # e4m3 for the DoubleRow chunks.
        xtr = xs.reshape(NT, P, KC, P).transpose(0, 3, 2, 1)  # [NT, hp, kh, tt]
        xtc = np.ascontiguousarray(xtr[:, :, :KM]).astype(
            ml_dtypes.bfloat16
        ).reshape(NT, P, KM * P)
        xtc8 = np.ascontiguousarray(xtr[:, :, KM:]).astype(
            ml_dtypes.float8_e4m3
        ).reshape(NT, P, (KC - KM) * P)
        in_maps.append(
            {"x": xs, "xtc": xtc, "xtc8": xtc8, "A": A_bf, "A8": A_f8, "c": c, "s": s}
        )
    return in_maps


def kernel(x, permute, gamma, beta, W, b):
    nc = _get_built()
    in_maps = _prep_inputs(x, permute, gamma, beta, W, b)
    res = run_bass_kernel_spmd(nc, in_maps, list(range(N_CORES))).results
    return np.concatenate([r["out"] for r in res], axis=0)


if __name__ == "__main__":
    rng = np.random.default_rng(0)
    x = rng.standard_normal((N_TOK, HIDDEN), dtype=np.float32)
    permute = rng.permutation(N_TOK).astype(np.int64)
    gamma = np.ones(HIDDEN, np.float32)
    beta = np.zeros(HIDDEN, np.float32)
    W = (rng.standard_normal((HIDDEN, HIDDEN), dtype=np.float32) / np.sqrt(HIDDEN))
    b = rng.standard_normal(HIDDEN, dtype=np.float32) * 0.01
    out = kernel(x=x, permute=permute, gamma=gamma, beta=beta, W=W, b=b)
    print(out.shape, out.dtype)
